# revision 24
# baseline (speedup 1.0000x reference)
"""Bass/Trainium2 kernel for the GaussianRecu (Kalman-style linear scan) model.

Reference recursion (C = I, dt = 0.01), per batch b, scanned over t:
    out_t   = dt * x_t                      (emitted before update)
    x_{t+1} = x_t + dt*(A - cov_t) x_t + cov_t dy_t
    cov_{t+1} = cov_t A + A cov_t

The cov recursion is linear with spectral radius 2*rho(A); for contracting A
it underflows to EXACT fp32 zero after a few dozen steps (t0 ~ 48 for the
reference inputs).  Once cov == 0 exactly the remaining recursion is exactly
x <- G x with G = I + dt*A, i.e.

    out[b, t, :] = dt * G^(t-t0) x*(b)

Device-side structure: partition p of an output tile holds timesteps
t = 512p + c, c in [0,512).  With z[b,p] = G^(512p - t0) x*(b) (host fp64)
and R[k, 2c+j] = dt * (G^c)[j,k] (host fp64), every output tile is a K=2
matmul on the otherwise-idle TensorEngine:

    out[b][p, f] = sum_k z[b,p,k] * R[k,f]
      = matmul(lhsT = z[.,p,k] as (K=2, M=128), rhs = R as (K=2, N))

Precision: inputs are bf16 hi+lo splits; the device accumulates
z_hi*R_hi + (z_hi*R_lo + z_lo*R_hi) in fp32 PSUM (the cross terms ride a
single K=4 matmul).  End-to-end rel err ~2e-5.

Per core this is a pure memory-roofline workload: ONE 36 KB parameter load,
~15us of K=2/K=4 matmuls + PSUM->SBUF copies (fully hidden), and 8.4 MB of
output DMA at the 16-SDMA-engine packet-rate ceiling (~420 GB/s).

Sharding: pure data parallel, batch 128 -> 16 rows per core on 8 cores.
"""

import numpy as np

B, T = 128, 65536
DT32 = np.float32(0.01)
N_CORES = 8
BPC = B // N_CORES  # 16 batch rows per core
P = 128             # SBUF partitions / coarse time blocks
C = T // P          # 512 timesteps per partition
F = 2 * C           # 1024 free-dim columns per out tile (c, j) pairs
ZCOLS = BPC * P     # 2048 z columns (b, p) pairs
W = ZCOLS + F       # 3072 packed zr columns

TRACE = False          # test harness may set True to collect a HW profile
LAST_RESULTS = None    # BassKernelResults of the most recent device run
_PROGRAMS = {}         # cached Bass program


def _build_program():
    """Device program: one packed bf16 load, 16 rank-2 matmul tiles.

    zr (6, W) bf16:
      rows [zh0, zh1, zh0, zh1, zl0, zl1], cols [0:ZCOLS)   z columns [k, b*128+p]
      rows [Rh0, Rh1, Rl0, Rl1, Rh0, Rh1], cols [ZCOLS:W)   R columns [k, f]

    so each tile is ONE K=6 matmul per 512-col PSUM bank:
      psum = [z_hi; z_hi; z_lo].T @ [R_hi; R_lo; R_hi]
           = z_hi.T R_hi + z_hi.T R_lo + z_lo.T R_hi      (fp32 in-array accum)

    The PE runs at the cold 1.2 GHz clock for these K-tiny matmuls (HAM
    never unthrottles), so minimizing MM count and hiding LDWEIGHTS is what
    matters: consecutive tiles alternate between array row offsets 0 and 32
    (the zr plane is loaded at both SBUF partition offsets) so tile b+1's
    LDWEIGHTS overlaps tile b's MATMUL (different row groups).

    PSUM->SBUF copies alternate DVE/ACT; 512KB stores stream on sync at the
    16-SDMA-engine packet-rate ceiling.  The first tile is quartered so the
    store stream starts early.
    """
    from contextlib import ExitStack

    import concourse.bacc as bacc
    import concourse.tile as tile
    from concourse import mybir

    f32 = mybir.dt.float32
    bf16 = mybir.dt.bfloat16
    nc = bacc.Bacc(
        "TRN2", target_bir_lowering=False, debug=False, num_devices=N_CORES
    )
    # zr carries the packed tables at partition rows 0-5 AND (duplicated)
    # 32-37, zeros between: one rectangular DMA fills both array row-group
    # offsets with a single completion receipt.
    zr = nc.declare_dram_parameter("zr", [38, W], bf16, isOutput=False)
    # bf16 output (host upcasts): halves the store stream; rel err ~2e-3
    # against the 2e-2 tolerance.
    out = nc.declare_dram_parameter("out", [BPC, P, F], bf16, isOutput=True)

    ctx = ExitStack()
    zrt = ctx.enter_context(nc.sbuf_tensor("zrt", [38, W], bf16))
    scratch = ctx.enter_context(nc.sbuf_tensor("scratch", [1, 8], f32))

    # Pre-TileContext: issue the (tiny) zr loads immediately after the sync
    # engine's preamble -- ahead of the Tile prologue barrier -- and trigger
    # the ACT table load now so neither sits on the critical path.  The zr
    # plane is loaded at SBUF partition offsets 0 and 32 so consecutive
    # tiles alternate array row groups (LDWEIGHTS overlaps the previous
    # MATMUL).  The PE blocks on the loads' completion sems before its first
    # LDWEIGHTS; every consumer instruction is behind the PE via the Tile
    # prologue barrier.
    ld_sem = nc.alloc_semaphore("zr_ld")
    nc.sync.dma_start(out=zrt[0:38, :], in_=zr[:]).then_inc(ld_sem, 16)
    nc.tensor.wait_ge(ld_sem, 16)

    with tile.TileContext(nc) as tc:
        with (
            tc.tile_pool(name="ot", bufs=8) as otp,
            tc.tile_pool(name="ps", bufs=4, space="PSUM") as psp,
        ):
            H = F // 2  # 512-col PSUM bank / matmul granularity
            for b in range(BPC):
                ps = psp.tile([P, F], f32)
                o = otp.tile([P, F], bf16)
                r0 = 32 * (b % 2)  # alternate array row groups
                lhs = zrt[r0 : r0 + 6, b * P : (b + 1) * P]
                for h in (0, H):
                    nc.tensor.matmul(
                        ps[:, h : h + H],
                        lhs,
                        zrt[r0 : r0 + 6, ZCOLS + h : ZCOLS + h + H],
                        start=True,
                        stop=True,
                    )
                # split the PSUM->SBUF cast-copy across both engines so the
                # tile's store is ready in one half-copy latency.  Tile 0
                # goes all-DVE: ACT's first op drags in ACT_TABLE_LOAD,
                # which then overlaps the early matmuls instead of gating
                # the first store.
                if b == 0:
                    nc.vector.tensor_copy(o[:, 0:H], ps[:, 0:H])
                    nc.vector.tensor_copy(o[:, H:F], ps[:, H:F])
                else:
                    nc.vector.tensor_copy(o[:, 0:H], ps[:, 0:H])
                    nc.scalar.copy(o[:, H:F], ps[:, H:F])
                if b in (0, BPC - 1):
                    # half stores at the stream edges: the first store
                    # starts one half earlier; the last store drains sooner
                    nc.sync.dma_start(out=out[b][:, 0:H], in_=o[:, 0:H])
                    nc.sync.dma_start(out=out[b][:, H:F], in_=o[:, H:F])
                else:
                    nc.sync.dma_start(out=out[b], in_=o[:])
    nc.compile()
    ctx.close()
    return nc


def _early_phase(dy, x0, cov0, A32):
    """Exact fp32 replica of the reference scan until cov == 0 exactly.

    Returns (early_out (B, t0, 2), xstar (B, 2), t0)."""
    x = x0.astype(np.float32).copy()
    cov = cov0.astype(np.float32).copy()
    rows = []
    t = 0
    while t < T and not np.all(cov == 0):
        rows.append(x * DT32)
        K = A32[None, :, :] - cov
        dx = np.einsum("bij,bj->bi", K, x) * DT32 + np.einsum(
            "bij,bj->bi", cov, dy[:, t, :]
        )
        cov = np.einsum("bij,jk->bik", cov, A32) + np.einsum(
            "ij,bjk->bik", A32, cov
        )
        x = x + dx
        t += 1
    early = (
        np.stack(rows, axis=1) if rows else np.zeros((B, 0, 2), np.float32)
    )
    return early.astype(np.float32), x, t


def _bf16_split(a):
    """hi/lo bf16 split of a float64 array (returned as bf16 ndarrays)."""
    import ml_dtypes

    bf = np.dtype(ml_dtypes.bfloat16)
    hi = a.astype(np.float32).astype(bf)
    lo = (a.astype(np.float32) - hi.astype(np.float32)).astype(bf)
    return hi, lo


def kernel(dy, x0, cov0, A):
    global LAST_RESULTS
    from concourse.bass_utils import run_bass_kernel_spmd
    import ml_dtypes

    bf = np.dtype(ml_dtypes.bfloat16)

    dy = np.ascontiguousarray(np.asarray(dy, dtype=np.float32))
    x0 = np.asarray(x0, dtype=np.float32)
    cov0 = np.asarray(cov0, dtype=np.float32)
    A32 = np.asarray(A, dtype=np.float32)
    assert dy.shape == (B, T, 2) and x0.shape == (B, 2)

    early, xstar, t0 = _early_phase(dy, x0, cov0, A32)

    # Host fp64 tables: G powers, per-(b,p) seeds z, per-c basis R.
    G = np.eye(2, dtype=np.float64) + float(DT32) * A32.astype(np.float64)
    Gc = np.empty((C, 2, 2), np.float64)
    cur = np.eye(2, dtype=np.float64)
    for c in range(C):
        Gc[c] = cur
        cur = cur @ G
    G512 = cur
    Rt = float(DT32) * np.transpose(Gc, (0, 2, 1))  # Rt[c][k,j] = dt*G^c[j,k]
    Rflat = Rt.transpose(1, 0, 2).reshape(2, F)  # R[k, 2c+j]

    H = np.empty((P, 2, 2), np.float64)
    h = np.linalg.matrix_power(np.linalg.inv(G), t0)
    for p in range(P):
        H[p] = h
        h = h @ G512
    z = np.einsum("pki,bi->bpk", H, xstar.astype(np.float64))  # (B, P, 2)

    z_hi, z_lo = _bf16_split(z)
    r_hi, r_lo = _bf16_split(Rflat)

    in_maps = []
    for r in range(N_CORES):
        zr_np = np.zeros((38, W), dtype=bf)
        zs = slice(r * BPC, (r + 1) * BPC)
        # z columns [k, b*128+p]: [z_hi; z_hi; z_lo]
        zh = z_hi[zs].reshape(ZCOLS, 2).T
        zr_np[0:2, 0:ZCOLS] = zh
        zr_np[2:4, 0:ZCOLS] = zh
        zr_np[4:6, 0:ZCOLS] = z_lo[zs].reshape(ZCOLS, 2).T
        # R columns [k, f]: [R_hi; R_lo; R_hi]
        zr_np[0:2, ZCOLS:W] = r_hi
        zr_np[2:4, ZCOLS:W] = r_lo
        zr_np[4:6, ZCOLS:W] = r_hi
        zr_np[32:38, :] = zr_np[0:6, :]  # duplicate at array row-group 32
        in_maps.append({"zr": np.ascontiguousarray(zr_np)})

    if "pe" not in _PROGRAMS:
        _PROGRAMS["pe"] = _build_program()
    nc = _PROGRAMS["pe"]

    res = run_bass_kernel_spmd(nc, in_maps, list(range(N_CORES)), trace=TRACE)
    LAST_RESULTS = res

    full = np.concatenate(
        [
            np.asarray(res.results[r]["out"])
            .astype(np.float32)
            .reshape(BPC, T, 2)
            for r in range(N_CORES)
        ],
        axis=0,
    )
    if t0 > 0:
        full[:, :t0, :] = early
    return np.ascontiguousarray(full.astype(np.float32, copy=False))


# revision 29
# speedup vs baseline: 1.2899x; 1.2899x over previous
"""Bass/Trainium2 kernel for the GaussianRecu (Kalman-style linear scan) model.

Reference recursion (C = I, dt = 0.01), per batch b, scanned over t:
    out_t   = dt * x_t                      (emitted before update)
    x_{t+1} = x_t + dt*(A - cov_t) x_t + cov_t dy_t
    cov_{t+1} = cov_t A + A cov_t

The cov recursion is linear with spectral radius 2*rho(A); for contracting A
it underflows to EXACT fp32 zero after a few dozen steps (t0 ~ 48 for the
reference inputs).  Once cov == 0 exactly the remaining recursion is exactly
x <- G x with G = I + dt*A, i.e.

    out[b, t, :] = dt * G^(t-t0) x*(b)

Device-side structure: partition p of an output tile holds timesteps
t = 512p + c, c in [0,512).  With z[b,p] = G^(512p - t0) x*(b) (host fp64)
and R[k, 2c+j] = dt * (G^c)[j,k] (host fp64), every output tile is a K=2
matmul on the otherwise-idle TensorEngine:

    out[b][p, f] = sum_k z[b,p,k] * R[k,f]
      = matmul(lhsT = z[.,p,k] as (K=2, M=128), rhs = R as (K=2, N))

Precision: inputs are bf16 hi+lo splits; the device accumulates
z_hi*R_hi + (z_hi*R_lo + z_lo*R_hi) in fp32 PSUM (the cross terms ride a
single K=4 matmul).  End-to-end rel err ~2e-5.

Per core this is a pure memory-roofline workload: ONE 36 KB parameter load,
~15us of K=2/K=4 matmuls + PSUM->SBUF copies (fully hidden), and 8.4 MB of
output DMA at the 16-SDMA-engine packet-rate ceiling (~420 GB/s).

Sharding: pure data parallel, batch 128 -> 16 rows per core on 8 cores.
"""

import numpy as np

B, T = 128, 65536
DT32 = np.float32(0.01)
N_CORES = 8
BPC = B // N_CORES  # 16 batch rows per core
P = 128             # SBUF partitions / coarse time blocks
C = T // P          # 512 timesteps per partition
F = 2 * C           # 1024 free-dim columns per out tile (c, j) pairs
ZCOLS = BPC * P     # 2048 z columns (b, p) pairs
W = ZCOLS + F       # 3072 packed zr columns

TRACE = False          # test harness may set True to collect a HW profile
LAST_RESULTS = None    # BassKernelResults of the most recent device run
_PROGRAMS = {}         # cached Bass program


def _build_program():
    """Device program: one packed bf16 load, 16 rank-2 matmul tiles.

    zr (6, W) bf16:
      rows [zh0, zh1, zh0, zh1, zl0, zl1], cols [0:ZCOLS)   z columns [k, b*128+p]
      rows [Rh0, Rh1, Rl0, Rl1, Rh0, Rh1], cols [ZCOLS:W)   R columns [k, f]

    so each tile is ONE K=6 matmul per 512-col PSUM bank:
      psum = [z_hi; z_hi; z_lo].T @ [R_hi; R_lo; R_hi]
           = z_hi.T R_hi + z_hi.T R_lo + z_lo.T R_hi      (fp32 in-array accum)

    The PE runs at the cold 1.2 GHz clock for these K-tiny matmuls (HAM
    never unthrottles), so minimizing MM count and hiding LDWEIGHTS is what
    matters: consecutive tiles alternate between array row offsets 0 and 32
    (the zr plane is loaded at both SBUF partition offsets) so tile b+1's
    LDWEIGHTS overlaps tile b's MATMUL (different row groups).

    PSUM->SBUF copies alternate DVE/ACT; 512KB stores stream on sync at the
    16-SDMA-engine packet-rate ceiling.  The first tile is quartered so the
    store stream starts early.
    """
    from contextlib import ExitStack

    import concourse.bacc as bacc
    import concourse.tile as tile
    from concourse import mybir

    f32 = mybir.dt.float32
    bf16 = mybir.dt.bfloat16
    nc = bacc.Bacc(
        "TRN2", target_bir_lowering=False, debug=False, num_devices=N_CORES
    )
    zr = nc.declare_dram_parameter("zr", [6, W], bf16, isOutput=False)
    # bf16 output (host upcasts): halves the store stream; rel err ~2e-3
    # against the 2e-2 tolerance.
    out = nc.declare_dram_parameter("out", [BPC, P, F], bf16, isOutput=True)

    ctx = ExitStack()
    zrt = ctx.enter_context(nc.sbuf_tensor("zrt", [38, W], bf16))
    scratch = ctx.enter_context(nc.sbuf_tensor("scratch", [1, 8], f32))

    # Pre-TileContext: issue the (tiny) zr loads immediately after the sync
    # engine's preamble -- ahead of the Tile prologue barrier -- and trigger
    # the ACT table load now so neither sits on the critical path.  The zr
    # plane is loaded at SBUF partition offsets 0 and 32 so consecutive
    # tiles alternate array row groups (LDWEIGHTS overlaps the previous
    # MATMUL).  The PE blocks on the loads' completion sems before its first
    # LDWEIGHTS; every consumer instruction is behind the PE via the Tile
    # prologue barrier.
    ld_sem = nc.alloc_semaphore("zr_ld")
    nc.scalar.dma_start(out=zrt[0:6, :], in_=zr[:]).then_inc(ld_sem, 16)
    nc.sync.dma_start(out=zrt[32:38, :], in_=zr[:]).then_inc(ld_sem, 16)
    nc.scalar.copy(scratch[0:1, 0:4], scratch[0:1, 4:8])  # pulls ACT_TABLE_LOAD early
    nc.tensor.wait_ge(ld_sem, 32)

    with tile.TileContext(nc) as tc:
        with (
            tc.tile_pool(name="ot", bufs=8) as otp,
            tc.tile_pool(name="ps", bufs=4, space="PSUM") as psp,
        ):
            H = F // 2  # 512-col PSUM bank / matmul granularity
            for b in range(BPC):
                ps = psp.tile([P, F], f32)
                o = otp.tile([P, F], bf16)
                r0 = 32 * (b % 2)  # alternate array row groups
                lhs = zrt[r0 : r0 + 6, b * P : (b + 1) * P]
                for h in (0, H):
                    nc.tensor.matmul(
                        ps[:, h : h + H],
                        lhs,
                        zrt[r0 : r0 + 6, ZCOLS + h : ZCOLS + h + H],
                        start=True,
                        stop=True,
                    )
                # split the PSUM->SBUF cast-copy across both engines so the
                # tile's store is ready in one half-copy latency
                nc.vector.tensor_copy(o[:, 0:H], ps[:, 0:H])
                nc.scalar.copy(o[:, H:F], ps[:, H:F])
                if b in (0, BPC - 1):
                    # half stores at the stream edges: the first store
                    # starts one half earlier; the last store drains sooner
                    nc.sync.dma_start(out=out[b][:, 0:H], in_=o[:, 0:H])
                    nc.sync.dma_start(out=out[b][:, H:F], in_=o[:, H:F])
                else:
                    nc.sync.dma_start(out=out[b], in_=o[:])
    nc.compile()
    ctx.close()
    return nc


def _early_phase(dy, x0, cov0, A32):
    """Exact fp32 replica of the reference scan until cov == 0 exactly.

    Returns (early_out (B, t0, 2), xstar (B, 2), t0)."""
    x = x0.astype(np.float32).copy()
    cov = cov0.astype(np.float32).copy()
    rows = []
    t = 0
    while t < T and not np.all(cov == 0):
        rows.append(x * DT32)
        K = A32[None, :, :] - cov
        dx = np.einsum("bij,bj->bi", K, x) * DT32 + np.einsum(
            "bij,bj->bi", cov, dy[:, t, :]
        )
        cov = np.einsum("bij,jk->bik", cov, A32) + np.einsum(
            "ij,bjk->bik", A32, cov
        )
        x = x + dx
        t += 1
    early = (
        np.stack(rows, axis=1) if rows else np.zeros((B, 0, 2), np.float32)
    )
    return early.astype(np.float32), x, t


def _bf16_split(a):
    """hi/lo bf16 split of a float64 array (returned as bf16 ndarrays)."""
    import ml_dtypes

    bf = np.dtype(ml_dtypes.bfloat16)
    hi = a.astype(np.float32).astype(bf)
    lo = (a.astype(np.float32) - hi.astype(np.float32)).astype(bf)
    return hi, lo


def kernel(dy, x0, cov0, A):
    global LAST_RESULTS
    from concourse.bass_utils import run_bass_kernel_spmd
    import ml_dtypes

    bf = np.dtype(ml_dtypes.bfloat16)

    dy = np.ascontiguousarray(np.asarray(dy, dtype=np.float32))
    x0 = np.asarray(x0, dtype=np.float32)
    cov0 = np.asarray(cov0, dtype=np.float32)
    A32 = np.asarray(A, dtype=np.float32)
    assert dy.shape == (B, T, 2) and x0.shape == (B, 2)

    early, xstar, t0 = _early_phase(dy, x0, cov0, A32)

    # Host fp64 tables: G powers, per-(b,p) seeds z, per-c basis R.
    G = np.eye(2, dtype=np.float64) + float(DT32) * A32.astype(np.float64)
    Gc = np.empty((C, 2, 2), np.float64)
    cur = np.eye(2, dtype=np.float64)
    for c in range(C):
        Gc[c] = cur
        cur = cur @ G
    G512 = cur
    Rt = float(DT32) * np.transpose(Gc, (0, 2, 1))  # Rt[c][k,j] = dt*G^c[j,k]
    Rflat = Rt.transpose(1, 0, 2).reshape(2, F)  # R[k, 2c+j]

    H = np.empty((P, 2, 2), np.float64)
    h = np.linalg.matrix_power(np.linalg.inv(G), t0)
    for p in range(P):
        H[p] = h
        h = h @ G512
    z = np.einsum("pki,bi->bpk", H, xstar.astype(np.float64))  # (B, P, 2)

    z_hi, z_lo = _bf16_split(z)
    r_hi, r_lo = _bf16_split(Rflat)

    in_maps = []
    for r in range(N_CORES):
        zr_np = np.zeros((6, W), dtype=bf)
        zs = slice(r * BPC, (r + 1) * BPC)
        # z columns [k, b*128+p]: [z_hi; z_hi; z_lo]
        zh = z_hi[zs].reshape(ZCOLS, 2).T
        zr_np[0:2, 0:ZCOLS] = zh
        zr_np[2:4, 0:ZCOLS] = zh
        zr_np[4:6, 0:ZCOLS] = z_lo[zs].reshape(ZCOLS, 2).T
        # R columns [k, f]: [R_hi; R_lo; R_hi]
        zr_np[0:2, ZCOLS:W] = r_hi
        zr_np[2:4, ZCOLS:W] = r_lo
        zr_np[4:6, ZCOLS:W] = r_hi
        in_maps.append({"zr": np.ascontiguousarray(zr_np)})

    if "pe" not in _PROGRAMS:
        _PROGRAMS["pe"] = _build_program()
    nc = _PROGRAMS["pe"]

    res = run_bass_kernel_spmd(nc, in_maps, list(range(N_CORES)), trace=TRACE)
    LAST_RESULTS = res

    full = np.concatenate(
        [
            np.asarray(res.results[r]["out"])
            .astype(np.float32)
            .reshape(BPC, T, 2)
            for r in range(N_CORES)
        ],
        axis=0,
    )
    if t0 > 0:
        full[:, :t0, :] = early
    return np.ascontiguousarray(full.astype(np.float32, copy=False))


# revision 30
# speedup vs baseline: 1.3145x; 1.0191x over previous
"""Bass/Trainium2 kernel for the GaussianRecu (Kalman-style linear scan) model.

Reference recursion (C = I, dt = 0.01), per batch b, scanned over t:
    out_t   = dt * x_t                      (emitted before update)
    x_{t+1} = x_t + dt*(A - cov_t) x_t + cov_t dy_t
    cov_{t+1} = cov_t A + A cov_t

The cov recursion is linear with spectral radius 2*rho(A); for contracting A
it underflows to EXACT fp32 zero after a few dozen steps (t0 ~ 48 for the
reference inputs).  Once cov == 0 exactly the remaining recursion is exactly
x <- G x with G = I + dt*A, i.e.

    out[b, t, :] = dt * G^(t-t0) x*(b)

Device-side structure: partition p of an output tile holds timesteps
t = 512p + c, c in [0,512).  With z[b,p] = G^(512p - t0) x*(b) (host fp64)
and R[k, 2c+j] = dt * (G^c)[j,k] (host fp64), every output tile is a K=2
matmul on the otherwise-idle TensorEngine:

    out[b][p, f] = sum_k z[b,p,k] * R[k,f]
      = matmul(lhsT = z[.,p,k] as (K=2, M=128), rhs = R as (K=2, N))

Precision: inputs are bf16 hi+lo splits; the device accumulates
z_hi*R_hi + (z_hi*R_lo + z_lo*R_hi) in fp32 PSUM (the cross terms ride a
single K=4 matmul).  End-to-end rel err ~2e-5.

Per core this is a pure memory-roofline workload: ONE 36 KB parameter load,
~15us of K=2/K=4 matmuls + PSUM->SBUF copies (fully hidden), and 8.4 MB of
output DMA at the 16-SDMA-engine packet-rate ceiling (~420 GB/s).

Sharding: pure data parallel, batch 128 -> 16 rows per core on 8 cores.
"""

import numpy as np

B, T = 128, 65536
DT32 = np.float32(0.01)
N_CORES = 8
BPC = B // N_CORES  # 16 batch rows per core
P = 128             # SBUF partitions / coarse time blocks
C = T // P          # 512 timesteps per partition
F = 2 * C           # 1024 free-dim columns per out tile (c, j) pairs
ZCOLS = BPC * P     # 2048 z columns (b, p) pairs
W = ZCOLS + F       # 3072 packed zr columns

TRACE = False          # test harness may set True to collect a HW profile
LAST_RESULTS = None    # BassKernelResults of the most recent device run
_PROGRAMS = {}         # cached Bass program


def _build_program():
    """Device program: one packed bf16 load, 16 rank-2 matmul tiles.

    zr (6, W) bf16:
      rows [zh0, zh1, zh0, zh1, zl0, zl1], cols [0:ZCOLS)   z columns [k, b*128+p]
      rows [Rh0, Rh1, Rl0, Rl1, Rh0, Rh1], cols [ZCOLS:W)   R columns [k, f]

    so each tile is ONE K=6 matmul per 512-col PSUM bank:
      psum = [z_hi; z_hi; z_lo].T @ [R_hi; R_lo; R_hi]
           = z_hi.T R_hi + z_hi.T R_lo + z_lo.T R_hi      (fp32 in-array accum)

    The PE runs at the cold 1.2 GHz clock for these K-tiny matmuls (HAM
    never unthrottles), so minimizing MM count and hiding LDWEIGHTS is what
    matters: consecutive tiles alternate between array row offsets 0 and 32
    (the zr plane is loaded at both SBUF partition offsets) so tile b+1's
    LDWEIGHTS overlaps tile b's MATMUL (different row groups).

    PSUM->SBUF copies alternate DVE/ACT; 512KB stores stream on sync at the
    16-SDMA-engine packet-rate ceiling.  The first tile is quartered so the
    store stream starts early.
    """
    from contextlib import ExitStack

    import concourse.bacc as bacc
    import concourse.tile as tile
    from concourse import mybir

    f32 = mybir.dt.float32
    bf16 = mybir.dt.bfloat16
    nc = bacc.Bacc(
        "TRN2", target_bir_lowering=False, debug=False, num_devices=N_CORES
    )
    zr = nc.declare_dram_parameter("zr", [6, W], bf16, isOutput=False)
    # bf16 output (host upcasts): halves the store stream; rel err ~2e-3
    # against the 2e-2 tolerance.
    out = nc.declare_dram_parameter("out", [BPC, P, F], bf16, isOutput=True)

    ctx = ExitStack()
    zrt = ctx.enter_context(nc.sbuf_tensor("zrt", [6, W], bf16))
    scratch = ctx.enter_context(nc.sbuf_tensor("scratch", [1, 8], f32))

    # Pre-TileContext: issue the (tiny) zr load immediately after the sync
    # engine's preamble -- ahead of the Tile prologue barrier -- and trigger
    # the ACT table load now so neither sits on the critical path.  The PE
    # blocks on the load's completion sem before its first LDWEIGHTS; every
    # consumer instruction is behind the PE via the Tile prologue barrier.
    ld_sem = nc.alloc_semaphore("zr_ld")
    nc.sync.dma_start(out=zrt[0:6, :], in_=zr[:]).then_inc(ld_sem, 16)
    nc.scalar.copy(scratch[0:1, 0:4], scratch[0:1, 4:8])  # pulls ACT_TABLE_LOAD early
    nc.tensor.wait_ge(ld_sem, 16)

    with tile.TileContext(nc) as tc:
        with (
            tc.tile_pool(name="zt", bufs=1) as ztp,
            tc.tile_pool(name="ot", bufs=8) as otp,
            tc.tile_pool(name="ps", bufs=4, space="PSUM") as psp,
        ):
            # Second copy of the zr plane at array row-group offset 32 so
            # consecutive tiles alternate row groups (LDWEIGHTS overlaps the
            # previous MATMUL).  Loaded as a Tile-tracked tile: odd tiles'
            # matmuls get exact waits on it without gating the PE start.
            zrt32 = ztp.tile([38, W], bf16)
            nc.sync.dma_start(out=zrt32[32:38, :], in_=zr[:])

            H = F // 2  # 512-col PSUM bank / matmul granularity
            for b in range(BPC):
                ps = psp.tile([P, F], f32)
                o = otp.tile([P, F], bf16)
                r0 = 32 * (b % 2)  # alternate array row groups
                src = zrt if b % 2 == 0 else zrt32
                lhs = src[r0 : r0 + 6, b * P : (b + 1) * P]
                for h in (0, H):
                    nc.tensor.matmul(
                        ps[:, h : h + H],
                        lhs,
                        src[r0 : r0 + 6, ZCOLS + h : ZCOLS + h + H],
                        start=True,
                        stop=True,
                    )
                # split the PSUM->SBUF cast-copy across both engines so the
                # tile's store is ready in one half-copy latency
                nc.vector.tensor_copy(o[:, 0:H], ps[:, 0:H])
                nc.scalar.copy(o[:, H:F], ps[:, H:F])
                if b in (0, BPC - 1):
                    # half stores at the stream edges: the first store
                    # starts one half earlier; the last store drains sooner
                    nc.sync.dma_start(out=out[b][:, 0:H], in_=o[:, 0:H])
                    nc.sync.dma_start(out=out[b][:, H:F], in_=o[:, H:F])
                else:
                    nc.sync.dma_start(out=out[b], in_=o[:])
    nc.compile()
    ctx.close()
    return nc


def _early_phase(dy, x0, cov0, A32):
    """Exact fp32 replica of the reference scan until cov == 0 exactly.

    Returns (early_out (B, t0, 2), xstar (B, 2), t0)."""
    x = x0.astype(np.float32).copy()
    cov = cov0.astype(np.float32).copy()
    rows = []
    t = 0
    while t < T and not np.all(cov == 0):
        rows.append(x * DT32)
        K = A32[None, :, :] - cov
        dx = np.einsum("bij,bj->bi", K, x) * DT32 + np.einsum(
            "bij,bj->bi", cov, dy[:, t, :]
        )
        cov = np.einsum("bij,jk->bik", cov, A32) + np.einsum(
            "ij,bjk->bik", A32, cov
        )
        x = x + dx
        t += 1
    early = (
        np.stack(rows, axis=1) if rows else np.zeros((B, 0, 2), np.float32)
    )
    return early.astype(np.float32), x, t


def _bf16_split(a):
    """hi/lo bf16 split of a float64 array (returned as bf16 ndarrays)."""
    import ml_dtypes

    bf = np.dtype(ml_dtypes.bfloat16)
    hi = a.astype(np.float32).astype(bf)
    lo = (a.astype(np.float32) - hi.astype(np.float32)).astype(bf)
    return hi, lo


def kernel(dy, x0, cov0, A):
    global LAST_RESULTS
    from concourse.bass_utils import run_bass_kernel_spmd
    import ml_dtypes

    bf = np.dtype(ml_dtypes.bfloat16)

    dy = np.ascontiguousarray(np.asarray(dy, dtype=np.float32))
    x0 = np.asarray(x0, dtype=np.float32)
    cov0 = np.asarray(cov0, dtype=np.float32)
    A32 = np.asarray(A, dtype=np.float32)
    assert dy.shape == (B, T, 2) and x0.shape == (B, 2)

    early, xstar, t0 = _early_phase(dy, x0, cov0, A32)

    # Host fp64 tables: G powers, per-(b,p) seeds z, per-c basis R.
    G = np.eye(2, dtype=np.float64) + float(DT32) * A32.astype(np.float64)
    Gc = np.empty((C, 2, 2), np.float64)
    cur = np.eye(2, dtype=np.float64)
    for c in range(C):
        Gc[c] = cur
        cur = cur @ G
    G512 = cur
    Rt = float(DT32) * np.transpose(Gc, (0, 2, 1))  # Rt[c][k,j] = dt*G^c[j,k]
    Rflat = Rt.transpose(1, 0, 2).reshape(2, F)  # R[k, 2c+j]

    H = np.empty((P, 2, 2), np.float64)
    h = np.linalg.matrix_power(np.linalg.inv(G), t0)
    for p in range(P):
        H[p] = h
        h = h @ G512
    z = np.einsum("pki,bi->bpk", H, xstar.astype(np.float64))  # (B, P, 2)

    z_hi, z_lo = _bf16_split(z)
    r_hi, r_lo = _bf16_split(Rflat)

    in_maps = []
    for r in range(N_CORES):
        zr_np = np.zeros((6, W), dtype=bf)
        zs = slice(r * BPC, (r + 1) * BPC)
        # z columns [k, b*128+p]: [z_hi; z_hi; z_lo]
        zh = z_hi[zs].reshape(ZCOLS, 2).T
        zr_np[0:2, 0:ZCOLS] = zh
        zr_np[2:4, 0:ZCOLS] = zh
        zr_np[4:6, 0:ZCOLS] = z_lo[zs].reshape(ZCOLS, 2).T
        # R columns [k, f]: [R_hi; R_lo; R_hi]
        zr_np[0:2, ZCOLS:W] = r_hi
        zr_np[2:4, ZCOLS:W] = r_lo
        zr_np[4:6, ZCOLS:W] = r_hi
        in_maps.append({"zr": np.ascontiguousarray(zr_np)})

    if "pe" not in _PROGRAMS:
        _PROGRAMS["pe"] = _build_program()
    nc = _PROGRAMS["pe"]

    res = run_bass_kernel_spmd(nc, in_maps, list(range(N_CORES)), trace=TRACE)
    LAST_RESULTS = res

    full = np.concatenate(
        [
            np.asarray(res.results[r]["out"])
            .astype(np.float32)
            .reshape(BPC, T, 2)
            for r in range(N_CORES)
        ],
        axis=0,
    )
    if t0 > 0:
        full[:, :t0, :] = early
    return np.ascontiguousarray(full.astype(np.float32, copy=False))


# revision 33
# speedup vs baseline: 1.4025x; 1.0669x over previous
"""Bass/Trainium2 kernel for the GaussianRecu (Kalman-style linear scan) model.

Reference recursion (C = I, dt = 0.01), per batch b, scanned over t:
    out_t   = dt * x_t                      (emitted before update)
    x_{t+1} = x_t + dt*(A - cov_t) x_t + cov_t dy_t
    cov_{t+1} = cov_t A + A cov_t

The cov recursion is linear with spectral radius 2*rho(A); for contracting A
it underflows to EXACT fp32 zero after a few dozen steps (t0 ~ 48 for the
reference inputs).  Once cov == 0 exactly the remaining recursion is exactly
x <- G x with G = I + dt*A, i.e.

    out[b, t, :] = dt * G^(t-t0) x*(b)

Device-side structure: partition p of an output tile holds timesteps
t = 512p + c, c in [0,512).  With z[b,p] = G^(512p - t0) x*(b) (host fp64)
and R[k, 2c+j] = dt * (G^c)[j,k] (host fp64), every output tile is a K=2
matmul on the otherwise-idle TensorEngine:

    out[b][p, f] = sum_k z[b,p,k] * R[k,f]
      = matmul(lhsT = z[.,p,k] as (K=2, M=128), rhs = R as (K=2, N))

Precision: inputs are bf16 hi+lo splits; the device accumulates
z_hi*R_hi + (z_hi*R_lo + z_lo*R_hi) in fp32 PSUM (the cross terms ride a
single K=4 matmul).  End-to-end rel err ~2e-5.

Per core this is a pure memory-roofline workload: ONE 36 KB parameter load,
~15us of K=2/K=4 matmuls + PSUM->SBUF copies (fully hidden), and 8.4 MB of
output DMA at the 16-SDMA-engine packet-rate ceiling (~420 GB/s).

Sharding: pure data parallel, batch 128 -> 16 rows per core on 8 cores.
"""

import numpy as np

B, T = 128, 65536
DT32 = np.float32(0.01)
N_CORES = 8
BPC = B // N_CORES  # 16 batch rows per core
P = 128             # SBUF partitions / coarse time blocks
C = T // P          # 512 timesteps per partition
F = 2 * C           # 1024 free-dim columns per out tile (c, j) pairs
ZCOLS = BPC * P     # 2048 z columns (b, p) pairs
W = ZCOLS + F       # 3072 packed zr columns

TRACE = False          # test harness may set True to collect a HW profile
LAST_RESULTS = None    # BassKernelResults of the most recent device run
_PROGRAMS = {}         # cached Bass program


def _build_program():
    """Device program: one packed bf16 load, 16 rank-2 matmul tiles.

    zr (6, W) bf16:
      rows [zh0, zh1, zh0, zh1, zl0, zl1], cols [0:ZCOLS)   z columns [k, b*128+p]
      rows [Rh0, Rh1, Rl0, Rl1, Rh0, Rh1], cols [ZCOLS:W)   R columns [k, f]

    so each tile is ONE K=6 matmul per 512-col PSUM bank:
      psum = [z_hi; z_hi; z_lo].T @ [R_hi; R_lo; R_hi]
           = z_hi.T R_hi + z_hi.T R_lo + z_lo.T R_hi      (fp32 in-array accum)

    The PE runs at the cold 1.2 GHz clock for these K-tiny matmuls (HAM
    never unthrottles), so minimizing MM count and hiding LDWEIGHTS is what
    matters: consecutive tiles alternate between array row offsets 0 and 32
    (the zr plane is loaded at both SBUF partition offsets) so tile b+1's
    LDWEIGHTS overlaps tile b's MATMUL (different row groups).

    PSUM->SBUF copies alternate DVE/ACT; 512KB stores stream on sync at the
    16-SDMA-engine packet-rate ceiling.  The first tile is quartered so the
    store stream starts early.
    """
    from contextlib import ExitStack

    import concourse.bacc as bacc
    import concourse.tile as tile
    from concourse import mybir

    f32 = mybir.dt.float32
    bf16 = mybir.dt.bfloat16
    nc = bacc.Bacc(
        "TRN2", target_bir_lowering=False, debug=False, num_devices=N_CORES
    )
    zr = nc.declare_dram_parameter("zr", [6, W], bf16, isOutput=False)
    # bf16 output (host upcasts): halves the store stream; rel err ~2e-3
    # against the 2e-2 tolerance.  Pair-major layout: two batches per store
    # keeps store descriptors at 4KB/partition and halves the issue count.
    out = nc.declare_dram_parameter("out", [BPC // 2, P, 2 * F], bf16, isOutput=True)

    ctx = ExitStack()
    zrt = ctx.enter_context(nc.sbuf_tensor("zrt", [6, W], bf16))
    scratch = ctx.enter_context(nc.sbuf_tensor("scratch", [1, 8], f32))

    # Pre-TileContext: issue the (tiny) zr load immediately after the sync
    # engine's preamble -- ahead of the Tile prologue barrier -- and trigger
    # the ACT table load now so neither sits on the critical path.  The PE
    # blocks on the load's completion sem before its first LDWEIGHTS; every
    # consumer instruction is behind the PE via the Tile prologue barrier.
    ld_sem = nc.alloc_semaphore("zr_ld")
    nc.sync.dma_start(out=zrt[0:6, :], in_=zr[:]).then_inc(ld_sem, 16)
    nc.scalar.copy(scratch[0:1, 0:4], scratch[0:1, 4:8])  # pulls ACT_TABLE_LOAD early
    nc.tensor.wait_ge(ld_sem, 16)

    with tile.TileContext(nc) as tc:
        with (
            tc.tile_pool(name="zt", bufs=1) as ztp,
            tc.tile_pool(name="ot", bufs=8) as otp,
            tc.tile_pool(name="ps", bufs=4, space="PSUM") as psp,
        ):
            # Second copy of the zr plane at array row-group offset 32 so
            # consecutive tiles alternate row groups (LDWEIGHTS overlaps the
            # previous MATMUL).  Loaded as a Tile-tracked tile: odd tiles'
            # matmuls get exact waits on it without gating the PE start.
            zrt32 = ztp.tile([38, W], bf16)
            nc.sync.dma_start(out=zrt32[32:38, :], in_=zr[:])

            H = F // 2  # 512-col PSUM bank / matmul granularity
            for i in range(BPC // 2):
                o = otp.tile([P, 2 * F], bf16)
                for j in (0, 1):
                    b = 2 * i + j
                    ps = psp.tile([P, F], f32)
                    r0 = 32 * (b % 2)  # alternate array row groups
                    src = zrt if b % 2 == 0 else zrt32
                    lhs = src[r0 : r0 + 6, b * P : (b + 1) * P]
                    for h in (0, H):
                        nc.tensor.matmul(
                            ps[:, h : h + H],
                            lhs,
                            src[r0 : r0 + 6, ZCOLS + h : ZCOLS + h + H],
                            start=True,
                            stop=True,
                        )
                    # per-batch PSUM->SBUF cast-copy, engines alternating
                    if j == 0:
                        nc.vector.tensor_copy(o[:, 0:F], ps[:])
                    else:
                        nc.scalar.copy(o[:, F : 2 * F], ps[:])
                nc.sync.dma_start(out=out[i], in_=o[:])
    nc.compile()
    ctx.close()
    return nc


def _early_phase(dy, x0, cov0, A32):
    """Exact fp32 replica of the reference scan until cov == 0 exactly.

    Returns (early_out (B, t0, 2), xstar (B, 2), t0)."""
    x = x0.astype(np.float32).copy()
    cov = cov0.astype(np.float32).copy()
    rows = []
    t = 0
    while t < T and not np.all(cov == 0):
        rows.append(x * DT32)
        K = A32[None, :, :] - cov
        dx = np.einsum("bij,bj->bi", K, x) * DT32 + np.einsum(
            "bij,bj->bi", cov, dy[:, t, :]
        )
        cov = np.einsum("bij,jk->bik", cov, A32) + np.einsum(
            "ij,bjk->bik", A32, cov
        )
        x = x + dx
        t += 1
    early = (
        np.stack(rows, axis=1) if rows else np.zeros((B, 0, 2), np.float32)
    )
    return early.astype(np.float32), x, t


def _bf16_split(a):
    """hi/lo bf16 split of a float64 array (returned as bf16 ndarrays)."""
    import ml_dtypes

    bf = np.dtype(ml_dtypes.bfloat16)
    hi = a.astype(np.float32).astype(bf)
    lo = (a.astype(np.float32) - hi.astype(np.float32)).astype(bf)
    return hi, lo


def kernel(dy, x0, cov0, A):
    global LAST_RESULTS
    from concourse.bass_utils import run_bass_kernel_spmd
    import ml_dtypes

    bf = np.dtype(ml_dtypes.bfloat16)

    dy = np.ascontiguousarray(np.asarray(dy, dtype=np.float32))
    x0 = np.asarray(x0, dtype=np.float32)
    cov0 = np.asarray(cov0, dtype=np.float32)
    A32 = np.asarray(A, dtype=np.float32)
    assert dy.shape == (B, T, 2) and x0.shape == (B, 2)

    early, xstar, t0 = _early_phase(dy, x0, cov0, A32)

    # Host fp64 tables: G powers, per-(b,p) seeds z, per-c basis R.
    G = np.eye(2, dtype=np.float64) + float(DT32) * A32.astype(np.float64)
    Gc = np.empty((C, 2, 2), np.float64)
    cur = np.eye(2, dtype=np.float64)
    for c in range(C):
        Gc[c] = cur
        cur = cur @ G
    G512 = cur
    Rt = float(DT32) * np.transpose(Gc, (0, 2, 1))  # Rt[c][k,j] = dt*G^c[j,k]
    Rflat = Rt.transpose(1, 0, 2).reshape(2, F)  # R[k, 2c+j]

    H = np.empty((P, 2, 2), np.float64)
    h = np.linalg.matrix_power(np.linalg.inv(G), t0)
    for p in range(P):
        H[p] = h
        h = h @ G512
    z = np.einsum("pki,bi->bpk", H, xstar.astype(np.float64))  # (B, P, 2)

    z_hi, z_lo = _bf16_split(z)
    r_hi, r_lo = _bf16_split(Rflat)

    in_maps = []
    for r in range(N_CORES):
        zr_np = np.zeros((6, W), dtype=bf)
        zs = slice(r * BPC, (r + 1) * BPC)
        # z columns [k, b*128+p]: [z_hi; z_hi; z_lo]
        zh = z_hi[zs].reshape(ZCOLS, 2).T
        zr_np[0:2, 0:ZCOLS] = zh
        zr_np[2:4, 0:ZCOLS] = zh
        zr_np[4:6, 0:ZCOLS] = z_lo[zs].reshape(ZCOLS, 2).T
        # R columns [k, f]: [R_hi; R_lo; R_hi]
        zr_np[0:2, ZCOLS:W] = r_hi
        zr_np[2:4, ZCOLS:W] = r_lo
        zr_np[4:6, ZCOLS:W] = r_hi
        in_maps.append({"zr": np.ascontiguousarray(zr_np)})

    if "pe" not in _PROGRAMS:
        _PROGRAMS["pe"] = _build_program()
    nc = _PROGRAMS["pe"]

    res = run_bass_kernel_spmd(nc, in_maps, list(range(N_CORES)), trace=TRACE)
    LAST_RESULTS = res

    # device layout (8, P, 2F): (pair, partition, batch-in-pair major cols)
    full = np.concatenate(
        [
            np.asarray(res.results[r]["out"])
            .astype(np.float32)
            .reshape(BPC // 2, P, 2, F)
            .transpose(0, 2, 1, 3)
            .reshape(BPC, T, 2)
            for r in range(N_CORES)
        ],
        axis=0,
    )
    if t0 > 0:
        full[:, :t0, :] = early
    return np.ascontiguousarray(full.astype(np.float32, copy=False))


# revision 34
# speedup vs baseline: 1.4141x; 1.0083x over previous
"""Bass/Trainium2 kernel for the GaussianRecu (Kalman-style linear scan) model.

Reference recursion (C = I, dt = 0.01), per batch b, scanned over t:
    out_t   = dt * x_t                      (emitted before update)
    x_{t+1} = x_t + dt*(A - cov_t) x_t + cov_t dy_t
    cov_{t+1} = cov_t A + A cov_t

The cov recursion is linear with spectral radius 2*rho(A); for contracting A
it underflows to EXACT fp32 zero after a few dozen steps (t0 ~ 48 for the
reference inputs).  Once cov == 0 exactly the remaining recursion is exactly
x <- G x with G = I + dt*A, i.e.

    out[b, t, :] = dt * G^(t-t0) x*(b)

Device-side structure: partition p of an output tile holds timesteps
t = 512p + c, c in [0,512).  With z[b,p] = G^(512p - t0) x*(b) (host fp64)
and R[k, 2c+j] = dt * (G^c)[j,k] (host fp64), every output tile is a K=2
matmul on the otherwise-idle TensorEngine:

    out[b][p, f] = sum_k z[b,p,k] * R[k,f]
      = matmul(lhsT = z[.,p,k] as (K=2, M=128), rhs = R as (K=2, N))

Precision: inputs are bf16 hi+lo splits; the device accumulates
z_hi*R_hi + (z_hi*R_lo + z_lo*R_hi) in fp32 PSUM (the cross terms ride a
single K=4 matmul).  End-to-end rel err ~2e-5.

Per core this is a pure memory-roofline workload: ONE 36 KB parameter load,
~15us of K=2/K=4 matmuls + PSUM->SBUF copies (fully hidden), and 8.4 MB of
output DMA at the 16-SDMA-engine packet-rate ceiling (~420 GB/s).

Sharding: pure data parallel, batch 128 -> 16 rows per core on 8 cores.
"""

import numpy as np

B, T = 128, 65536
DT32 = np.float32(0.01)
N_CORES = 8
BPC = B // N_CORES  # 16 batch rows per core
P = 128             # SBUF partitions / coarse time blocks
C = T // P          # 512 timesteps per partition
F = 2 * C           # 1024 free-dim columns per out tile (c, j) pairs
ZCOLS = BPC * P     # 2048 z columns (b, p) pairs
W = ZCOLS + F       # 3072 packed zr columns

TRACE = False          # test harness may set True to collect a HW profile
LAST_RESULTS = None    # BassKernelResults of the most recent device run
_PROGRAMS = {}         # cached Bass program


def _build_program():
    """Device program: one packed bf16 load, 16 rank-2 matmul tiles.

    zr (6, W) bf16:
      rows [zh0, zh1, zh0, zh1, zl0, zl1], cols [0:ZCOLS)   z columns [k, b*128+p]
      rows [Rh0, Rh1, Rl0, Rl1, Rh0, Rh1], cols [ZCOLS:W)   R columns [k, f]

    so each tile is ONE K=6 matmul per 512-col PSUM bank:
      psum = [z_hi; z_hi; z_lo].T @ [R_hi; R_lo; R_hi]
           = z_hi.T R_hi + z_hi.T R_lo + z_lo.T R_hi      (fp32 in-array accum)

    The PE runs at the cold 1.2 GHz clock for these K-tiny matmuls (HAM
    never unthrottles), so minimizing MM count and hiding LDWEIGHTS is what
    matters: consecutive tiles alternate between array row offsets 0 and 32
    (the zr plane is loaded at both SBUF partition offsets) so tile b+1's
    LDWEIGHTS overlaps tile b's MATMUL (different row groups).

    PSUM->SBUF copies alternate DVE/ACT; 512KB stores stream on sync at the
    16-SDMA-engine packet-rate ceiling.  The first tile is quartered so the
    store stream starts early.
    """
    from contextlib import ExitStack

    import concourse.bacc as bacc
    import concourse.tile as tile
    from concourse import mybir

    f32 = mybir.dt.float32
    bf16 = mybir.dt.bfloat16
    nc = bacc.Bacc(
        "TRN2", target_bir_lowering=False, debug=False, num_devices=N_CORES
    )
    zr = nc.declare_dram_parameter("zr", [6, W], bf16, isOutput=False)
    # bf16 output (host upcasts): halves the store stream; rel err ~2e-3
    # against the 2e-2 tolerance.  Pair-major layout: two batches per store
    # keeps store descriptors at 4KB/partition and halves the issue count.
    out = nc.declare_dram_parameter("out", [BPC // 2, P, 2 * F], bf16, isOutput=True)

    ctx = ExitStack()
    zrt = ctx.enter_context(nc.sbuf_tensor("zrt", [6, W], bf16))
    scratch = ctx.enter_context(nc.sbuf_tensor("scratch", [1, 8], f32))

    # Pre-TileContext: issue the (tiny) zr load immediately after the sync
    # engine's preamble -- ahead of the Tile prologue barrier -- and trigger
    # the ACT table load now so neither sits on the critical path.  The PE
    # blocks on the load's completion sem before its first LDWEIGHTS; every
    # consumer instruction is behind the PE via the Tile prologue barrier.
    ld_sem = nc.alloc_semaphore("zr_ld")
    nc.sync.dma_start(out=zrt[0:6, :], in_=zr[:]).then_inc(ld_sem, 16)
    nc.scalar.copy(scratch[0:1, 0:4], scratch[0:1, 4:8])  # pulls ACT_TABLE_LOAD early
    nc.tensor.wait_ge(ld_sem, 16)

    with tile.TileContext(nc) as tc:
        with (
            tc.tile_pool(name="zt", bufs=1) as ztp,
            tc.tile_pool(name="ot", bufs=8) as otp,
            tc.tile_pool(name="ps", bufs=4, space="PSUM") as psp,
        ):
            # Second copy of the zr plane at array row-group offset 32 so
            # consecutive tiles alternate row groups (LDWEIGHTS overlaps the
            # previous MATMUL).  Loaded as a Tile-tracked tile: odd tiles'
            # matmuls get exact waits on it without gating the PE start.
            zrt32 = ztp.tile([38, W], bf16)
            nc.sync.dma_start(out=zrt32[32:38, :], in_=zr[:])

            H = F // 2  # 512-col PSUM bank / matmul granularity
            for i in range(BPC // 2):
                o = otp.tile([P, 2 * F], bf16)
                for j in (0, 1):
                    b = 2 * i + j
                    ps = psp.tile([P, F], f32)
                    # alternate array row groups per half-matmul: every
                    # consecutive MATMUL is on a different row group, so
                    # each LDWEIGHTS overlaps the in-flight MATMUL
                    for h in (0, H):
                        r0 = 0 if h == 0 else 32
                        src = zrt if h == 0 else zrt32
                        nc.tensor.matmul(
                            ps[:, h : h + H],
                            src[r0 : r0 + 6, b * P : (b + 1) * P],
                            src[r0 : r0 + 6, ZCOLS + h : ZCOLS + h + H],
                            start=True,
                            stop=True,
                        )
                    # per-batch PSUM->SBUF cast-copy, engines alternating
                    if j == 0:
                        nc.vector.tensor_copy(o[:, 0:F], ps[:])
                    else:
                        nc.scalar.copy(o[:, F : 2 * F], ps[:])
                nc.sync.dma_start(out=out[i], in_=o[:])
    nc.compile()
    ctx.close()
    return nc


def _early_phase(dy, x0, cov0, A32):
    """Exact fp32 replica of the reference scan until cov == 0 exactly.

    Returns (early_out (B, t0, 2), xstar (B, 2), t0)."""
    x = x0.astype(np.float32).copy()
    cov = cov0.astype(np.float32).copy()
    rows = []
    t = 0
    while t < T and not np.all(cov == 0):
        rows.append(x * DT32)
        K = A32[None, :, :] - cov
        dx = np.einsum("bij,bj->bi", K, x) * DT32 + np.einsum(
            "bij,bj->bi", cov, dy[:, t, :]
        )
        cov = np.einsum("bij,jk->bik", cov, A32) + np.einsum(
            "ij,bjk->bik", A32, cov
        )
        x = x + dx
        t += 1
    early = (
        np.stack(rows, axis=1) if rows else np.zeros((B, 0, 2), np.float32)
    )
    return early.astype(np.float32), x, t


def _bf16_split(a):
    """hi/lo bf16 split of a float64 array (returned as bf16 ndarrays)."""
    import ml_dtypes

    bf = np.dtype(ml_dtypes.bfloat16)
    hi = a.astype(np.float32).astype(bf)
    lo = (a.astype(np.float32) - hi.astype(np.float32)).astype(bf)
    return hi, lo


def kernel(dy, x0, cov0, A):
    global LAST_RESULTS
    from concourse.bass_utils import run_bass_kernel_spmd
    import ml_dtypes

    bf = np.dtype(ml_dtypes.bfloat16)

    dy = np.ascontiguousarray(np.asarray(dy, dtype=np.float32))
    x0 = np.asarray(x0, dtype=np.float32)
    cov0 = np.asarray(cov0, dtype=np.float32)
    A32 = np.asarray(A, dtype=np.float32)
    assert dy.shape == (B, T, 2) and x0.shape == (B, 2)

    early, xstar, t0 = _early_phase(dy, x0, cov0, A32)

    # Host fp64 tables: G powers, per-(b,p) seeds z, per-c basis R.
    G = np.eye(2, dtype=np.float64) + float(DT32) * A32.astype(np.float64)
    Gc = np.empty((C, 2, 2), np.float64)
    cur = np.eye(2, dtype=np.float64)
    for c in range(C):
        Gc[c] = cur
        cur = cur @ G
    G512 = cur
    Rt = float(DT32) * np.transpose(Gc, (0, 2, 1))  # Rt[c][k,j] = dt*G^c[j,k]
    Rflat = Rt.transpose(1, 0, 2).reshape(2, F)  # R[k, 2c+j]

    H = np.empty((P, 2, 2), np.float64)
    h = np.linalg.matrix_power(np.linalg.inv(G), t0)
    for p in range(P):
        H[p] = h
        h = h @ G512
    z = np.einsum("pki,bi->bpk", H, xstar.astype(np.float64))  # (B, P, 2)

    z_hi, z_lo = _bf16_split(z)
    r_hi, r_lo = _bf16_split(Rflat)

    in_maps = []
    for r in range(N_CORES):
        zr_np = np.zeros((6, W), dtype=bf)
        zs = slice(r * BPC, (r + 1) * BPC)
        # z columns [k, b*128+p]: [z_hi; z_hi; z_lo]
        zh = z_hi[zs].reshape(ZCOLS, 2).T
        zr_np[0:2, 0:ZCOLS] = zh
        zr_np[2:4, 0:ZCOLS] = zh
        zr_np[4:6, 0:ZCOLS] = z_lo[zs].reshape(ZCOLS, 2).T
        # R columns [k, f]: [R_hi; R_lo; R_hi]
        zr_np[0:2, ZCOLS:W] = r_hi
        zr_np[2:4, ZCOLS:W] = r_lo
        zr_np[4:6, ZCOLS:W] = r_hi
        in_maps.append({"zr": np.ascontiguousarray(zr_np)})

    if "pe" not in _PROGRAMS:
        _PROGRAMS["pe"] = _build_program()
    nc = _PROGRAMS["pe"]

    res = run_bass_kernel_spmd(nc, in_maps, list(range(N_CORES)), trace=TRACE)
    LAST_RESULTS = res

    # device layout (8, P, 2F): (pair, partition, batch-in-pair major cols)
    full = np.concatenate(
        [
            np.asarray(res.results[r]["out"])
            .astype(np.float32)
            .reshape(BPC // 2, P, 2, F)
            .transpose(0, 2, 1, 3)
            .reshape(BPC, T, 2)
            for r in range(N_CORES)
        ],
        axis=0,
    )
    if t0 > 0:
        full[:, :t0, :] = early
    return np.ascontiguousarray(full.astype(np.float32, copy=False))
